# revision 19
# baseline (speedup 1.0000x reference)
"""DepthDC3x3 dynamic depthwise conv — Trainium2 Bass kernel, 8 NeuronCores.

Per-sample pipeline (data-parallel over batch N=8 -> one sample per core):
  A  = conv3x3(y, w_gk1) + b_gk1                    # [64,128,128]
  K  = conv1x1(A, w_gk2) + b_gk2                    # [576,128,128] -> per-tap
  r  = sum_k K[:,k] * shift_k(x)                    # dynamic depthwise 3x3
  out= conv3x3(r, w_fuse) + b_fuse                  # [64,128,128]

Key design points (v7, "all-normal"):
  * Half-split layout: SBUF partitions = (channel 0..63) x (image half).
  * Inputs are pre-padded + pre-cast to bf16 on the HOST into the exact
    on-chip layout ([128 part, 66, 130]); chunked HWDGE loads (y on the
    sync ring, x on the scalar ring) feed compute directly.
  * conv1 window-pairs produce one normal + one halves-swapped ("inverted")
    PSUM tile -> a2 windows alternate normal/inverted. A cheap SBUF->SBUF
    swap DMA per pair builds a2s (the opposite orientation), so every
    window is available in BOTH orientations.
  * Stage C uses the normal-orientation source for psA quadrants and the
    inverted source for psB quadrants -> ALL conv2 outputs land normal.
    Single accumulation chain (9 muls + 8 adds) writes straight into the
    one padded accumulator rpn. No rpi, no xswp.
  * conv3 is a single 9-tap pass over rpn (half the v6 PE work), one ACT
    eviction, no DVE add.
  * bf16 matmuls, 2x2 tile_position quadrants -> 4x PE concurrency.
"""

import threading

import numpy as np
import ml_dtypes

import concourse.bacc as bacc
import concourse.mybir as mybir
from concourse.tile import TileContext

BF = mybir.dt.bfloat16
F32 = mybir.dt.float32

C = 64            # channels
HR = 64           # rows per half
W = 128           # image width
PW = W + 2        # padded width
PR = HR + 2       # padded rows per half
NWIN = 16         # windows per half (4 rows x 128 cols each)
WPX = 512
RW = 4            # rows per window
KS = 3

# row chunks of the padded-row index space (0..66)
CHUNKS = ((0, 18), (18, 34), (34, 50), (50, 66))


_lock = threading.Lock()
_cache = {}


def _tap_off(t):
    return divmod(t, KS)


def build_nc():
    nc = bacc.Bacc("TRN2", target_bir_lowering=False, debug=False)

    xp_d = nc.dram_tensor("xpd", [128, PR, PW], BF, kind="ExternalInput")
    yp_d = nc.dram_tensor("ypd", [128, PR, PW], BF, kind="ExternalInput")
    wall_d = nc.dram_tensor("wall", [128, 27 * C], BF, kind="ExternalInput")
    ball_d = nc.dram_tensor("ball", [128, 11], F32, kind="ExternalInput")
    out_d = nc.dram_tensor("out", [C, 2 * HR, W], F32, kind="ExternalOutput")

    with TileContext(nc) as tc:
        with (
            tc.tile_pool(name="big", bufs=1) as big,
            tc.tile_pool(name="wpool", bufs=1) as wpool,
            tc.tile_pool(name="kev", bufs=6) as kevp,
            tc.tile_pool(name="prod", bufs=6) as prodp,
            tc.tile_pool(name="accs", bufs=4) as accp,
            tc.tile_pool(name="stag", bufs=3) as stagp,
            tc.tile_pool(name="ps13", bufs=2, space="PSUM") as ps13p,
            tc.tile_pool(name="psC", bufs=2, space="PSUM") as psCp,
        ):
            ypad = big.tile([128, PR, PW], BF)
            xpad = big.tile([128, PR, PW], BF)
            a2 = big.tile([128, NWIN * WPX], BF)
            a2s = big.tile([128, NWIN * WPX], BF)
            rpn = big.tile([128, PR, PW], BF)

            wall = wpool.tile([128, 27 * C], BF)
            ball = wpool.tile([128, 11], F32)
            W2O, W3O = 9 * C, 18 * C

            def ldchunk(dst, src, k, q):
                r0, r1 = CHUNKS[k]
                q.dma_start(out=dst[:, r0:r1, :], in_=src[:, r0:r1, :])

            ident = mybir.ActivationFunctionType.Identity

            def conv9(src, woff, psLR, w0):
                """3x3 conv quadrants for adjacent window-pair (w0, w0+1).
                psLR [128,1024]: [:,0:512] win w0 normal; [:,512:1024] win
                w0+1 inverted."""
                r0, r1 = RW * w0, RW * (w0 + 1)
                for t in range(9):
                    dy, dx = _tap_off(t)
                    st, sp = (t == 0), (t == 8)
                    lhsL = wall[0:64, woff + t * C:woff + (t + 1) * C]
                    lhsH = wall[64:128, woff + t * C:woff + (t + 1) * C]
                    nc.tensor.matmul(psLR[0:64, 0:512], lhsL,
                                     src[0:64, r0 + dy:r0 + dy + 4, dx:dx + 128],
                                     start=st, stop=sp, tile_position=(0, 0),
                                     skip_group_check=True)
                    nc.tensor.matmul(psLR[64:128, 0:512], lhsH,
                                     src[64:128, r0 + dy:r0 + dy + 4, dx:dx + 128],
                                     start=st, stop=sp, tile_position=(64, 64),
                                     skip_group_check=True)
                    nc.tensor.matmul(psLR[64:128, 512:1024], lhsL,
                                     src[0:64, r1 + dy:r1 + dy + 4, dx:dx + 128],
                                     start=st, stop=sp, tile_position=(0, 64),
                                     skip_group_check=True)
                    nc.tensor.matmul(psLR[0:64, 512:1024], lhsH,
                                     src[64:128, r1 + dy:r1 + dy + 4, dx:dx + 128],
                                     start=st, stop=sp, tile_position=(64, 0),
                                     skip_group_check=True)

            def conv1_pair(p):
                w0 = 2 * p
                psLR = ps13p.tile([128, 2 * WPX], F32, tag="ps13",
                                  name=f"c1ps{p}")
                conv9(ypad, 0, psLR, w0)
                nc.scalar.activation(a2[:, w0 * WPX:(w0 + 2) * WPX],
                                     psLR[:, :], ident, bias=ball[:, 0:1])

            def swp(p):
                """halves-swapped copy of a2 windows (2p, 2p+1) -> a2s.
                gpsimd (SWDGE) ring: fires the moment the conv1 eviction
                lands, without queueing behind the y/x bulk loads."""
                sl = slice(2 * p * WPX, (2 * p + 2) * WPX)
                nc.sync.dma_start(out=a2s[0:64, sl], in_=a2[64:128, sl])
                nc.sync.dma_start(out=a2s[64:128, sl], in_=a2[0:64, sl])

            def stage_q(q, cbs=None):
                """conv2 + dynamic multiply-sum for quad q = windows
                4q..4q+3 (16 consecutive rows). Per tap-pair jp: 4 PSUM
                tiles (2 per tap) -> 4 ACT evicts into two [128,2048] kev
                tiles -> 2 fused muls into a [128,4096] product -> 1 fused
                add into the [128,4096] accumulator. Fold + two final adds
                (row halves, for finer conv3 deps) write rpn.
                cbs = {jp: callback} sprinkles filler emission (conv1 /
                conv3 pairs) between jp groups."""
                w0 = 4 * q
                r0 = RW * w0
                acc_t = None
                prod8 = None

                def win_src(wv, inv):
                    # normal orientation lives in a2 for even windows and
                    # in a2s for odd ones; inv selects the opposite.
                    s = a2 if (wv % 2 == 0) != inv else a2s
                    return s[:, wv * WPX:(wv + 1) * WPX]

                def mm_tap(T1, T2, ll, lh, inv):
                    for wi in range(4):
                        src = win_src(w0 + wi, inv)
                        T = T1 if wi < 2 else T2
                        sl = slice((wi % 2) * WPX, (wi % 2 + 1) * WPX)
                        if not inv:
                            nc.tensor.matmul(T[0:64, sl], ll, src[0:64, :],
                                             tile_position=(0, 0),
                                             skip_group_check=True)
                            nc.tensor.matmul(T[64:128, sl], lh,
                                             src[64:128, :],
                                             tile_position=(64, 64),
                                             skip_group_check=True)
                        else:
                            nc.tensor.matmul(T[64:128, sl], ll, src[0:64, :],
                                             tile_position=(0, 64),
                                             skip_group_check=True)
                            nc.tensor.matmul(T[0:64, sl], lh,
                                             src[64:128, :],
                                             tile_position=(64, 0),
                                             skip_group_check=True)

                def xview(t):
                    dy, dx = _tap_off(t)
                    return xpad[:, r0 + dy:r0 + dy + 16, dx:dx + 128]

                def r3(flat2048):
                    return flat2048.rearrange("p (r c) -> p r c", c=128)

                for jp in range(5):
                    ta = 2 * jp
                    single = (ta == 8)
                    lhsA_l = wall[0:64, W2O + ta * C:W2O + (ta + 1) * C]
                    lhsA_h = wall[64:128, W2O + ta * C:W2O + (ta + 1) * C]
                    TA = psCp.tile([128, 2 * WPX], F32, tag="ps2",
                                   name=f"qA{q}_{jp}")
                    TA2 = psCp.tile([128, 2 * WPX], F32, tag="ps2",
                                    name=f"qA2{q}_{jp}")
                    mm_tap(TA, TA2, lhsA_l, lhsA_h, False)
                    kevA = kevp.tile([128, 4 * WPX], BF, tag="kev",
                                     name=f"kA{q}_{jp}")
                    nc.scalar.activation(kevA[:, 0:2 * WPX], TA[:, :], ident,
                                         bias=ball[:, 1 + ta:2 + ta])
                    nc.scalar.activation(kevA[:, 2 * WPX:4 * WPX], TA2[:, :],
                                         ident, bias=ball[:, 1 + ta:2 + ta])
                    if not single:
                        tb = ta + 1
                        lhsB_l = wall[0:64, W2O + tb * C:W2O + (tb + 1) * C]
                        lhsB_h = wall[64:128, W2O + tb * C:W2O + (tb + 1) * C]
                        TB = psCp.tile([128, 2 * WPX], F32, tag="ps2",
                                       name=f"qB{q}_{jp}")
                        TB2 = psCp.tile([128, 2 * WPX], F32, tag="ps2",
                                        name=f"qB2{q}_{jp}")
                        mm_tap(TB, TB2, lhsB_l, lhsB_h, True)
                        kevB = kevp.tile([128, 4 * WPX], BF, tag="kev",
                                         name=f"kB{q}_{jp}")
                        nc.scalar.activation(kevB[:, 0:2 * WPX], TB[:, :],
                                             ident,
                                             bias=ball[:, 1 + tb:2 + tb])
                        nc.scalar.activation(kevB[:, 2 * WPX:4 * WPX],
                                             TB2[:, :], ident,
                                             bias=ball[:, 1 + tb:2 + tb])

                    if jp == 0:
                        acc_t = accp.tile([128, 8 * WPX], BF, tag="acc",
                                          name=f"qac{q}")
                        tgt = acc_t
                    elif not single:
                        tgt = prodp.tile([128, 8 * WPX], BF, tag="prod",
                                         name=f"qpr{q}_{jp}")
                    else:
                        prod8 = prodp.tile([128, 8 * WPX], BF, tag="prod",
                                           name=f"qp8{q}")
                        tgt = prod8
                    nc.vector.tensor_mul(out=r3(tgt[:, 0:4 * WPX]),
                                         in0=r3(kevA[:, :]), in1=xview(ta))
                    if not single:
                        nc.vector.tensor_mul(
                            out=r3(tgt[:, 4 * WPX:8 * WPX]),
                            in0=r3(kevB[:, :]), in1=xview(tb))
                        if jp > 0:
                            nc.vector.tensor_add(out=acc_t[:, :],
                                                 in0=acc_t[:, :],
                                                 in1=tgt[:, :])
                    if cbs and jp in cbs:
                        cbs[jp]()

                # fold tap-B half into tap-A half, then add prod8 and write
                # rpn in two row-halves (windows w0,w0+1 | w0+2,w0+3) so
                # dependent conv3 pairs unlock as early as possible.
                nc.vector.tensor_add(out=acc_t[:, 0:4 * WPX],
                                     in0=acc_t[:, 0:4 * WPX],
                                     in1=acc_t[:, 4 * WPX:8 * WPX])
                for h in range(2):
                    sl = slice(h * 2 * WPX, (h + 1) * 2 * WPX)
                    rows = slice(r0 + 1 + 8 * h, r0 + 9 + 8 * h)
                    nc.vector.tensor_add(
                        out=rpn[:, rows, 1:129],
                        in0=r3(acc_t[:, sl]),
                        in1=r3(prod8[:, sl]))

            def halosA():
                # available after stage_c(0): rpn h0 bottom halo row.
                nc.sync.dma_start(out=rpn[0:64, 65:66, :],
                                  in_=rpn[64:128, 1:2, :])

            def halosB():
                # available after stage_c(13): rpn h1 top halo row
                nc.sync.dma_start(out=rpn[64:128, 0:1, :],
                                  in_=rpn[0:64, 64:65, :])

            # conv3: single 9-tap pass over rpn for windows (2p, 2p+1),
            # one ACT eviction, 4 stores.
            c3ps = {}

            def c3n(p, pool=ps13p, tg="ps13"):
                psN = pool.tile([128, 2 * WPX], F32, tag=tg, name=f"c3n{p}")
                conv9(rpn, W3O, psN, 2 * p)
                c3ps[p] = psN

            def c3fin(p):
                psN = c3ps.pop(p)
                w0 = 2 * p
                st_t = stagp.tile([128, 2 * WPX], F32, tag="stag",
                                  name=f"st{p}")
                nc.scalar.activation(st_t[:, :], psN[:, :], ident,
                                     bias=ball[:, 10:11])
                ra, rb = RW * w0, RW * (w0 + 1)
                # last pair: split stores across both rings to halve the
                # final drain (scalar engine is idle by then)
                q2 = nc.scalar if p == 7 else nc.sync
                nc.sync.dma_start(out=out_d[:, ra:ra + 4, :],
                                  in_=st_t[0:64, 0:512])
                q2.dma_start(out=out_d[:, HR + ra:HR + ra + 4, :],
                             in_=st_t[64:128, 0:512])
                nc.sync.dma_start(out=out_d[:, HR + rb:HR + rb + 4, :],
                                  in_=st_t[0:64, 512:1024])
                q2.dma_start(out=out_d[:, rb:rb + 4, :],
                             in_=st_t[64:128, 512:1024])

            def conv3_pair(p, pool=ps13p, tg="ps13"):
                c3n(p, pool, tg)
                c3fin(p)

            # ---- emission schedule ----
            # PE HAM warm-up: 64 dummy matmuls spread across all 4 quadrants
            # (4-way concurrent) flip the clock gate to 8/8 during the load
            # window; results are discarded. Source rows are zeroed first so
            # the reads are initialized (keeps CoreSim usable).
            nc.vector.memset(rpn[:, 20:26, :], 0.0)
            nc.vector.memset(rpn[:, 30:36, :], 0.0)
            psW = psCp.tile([128, 2 * WPX], F32, tag="ps2", name="warm")
            wq = (
                (psW[0:64, 0:512], rpn[0:64, 20:21, 1:65],
                 rpn[0:64, 22:26, 1:129], (0, 0)),
                (psW[64:128, 0:512], rpn[64:128, 20:21, 1:65],
                 rpn[64:128, 22:26, 1:129], (64, 64)),
                (psW[64:128, 512:1024], rpn[0:64, 30:31, 1:65],
                 rpn[0:64, 32:36, 1:129], (0, 64)),
                (psW[0:64, 512:1024], rpn[64:128, 30:31, 1:65],
                 rpn[64:128, 32:36, 1:129], (64, 0)),
            )
            for i in range(64):
                o, l, r, tp = wq[i % 4]
                nc.tensor.matmul(o, l, r, tile_position=tp,
                                 skip_group_check=True)

            # weights on the scalar ring: they load in parallel with y0a
            # instead of serializing between the y chunks on the sync ring
            nc.scalar.dma_start(out=wall[:], in_=wall_d[:])
            nc.scalar.dma_start(out=ball[:], in_=ball_d[:])
            nc.sync.dma_start(out=ypad[:, 0:10, :], in_=yp_d[:, 0:10, :])
            nc.sync.dma_start(out=ypad[:, 10:18, :], in_=yp_d[:, 10:18, :])

            nc.vector.memset(rpn[0:64, 0:1, :], 0.0)
            nc.vector.memset(rpn[64:128, 65:66, :], 0.0)
            nc.vector.memset(rpn[:, :, 0:1], 0.0)
            nc.vector.memset(rpn[:, :, 129:130], 0.0)

            # conv1 consumes pairs 0,1,6,7 first (y chunks 2,3 before 1);
            # x chunks follow the stage_c order 0,13,1,12,...
            for k in (2, 3, 1):
                ldchunk(ypad, yp_d, k, nc.sync)
            for k in (0, 3, 1, 2):
                ldchunk(xpad, xp_d, k, nc.scalar)

            # stage_q order 0,3,1,2: quad 3 early gates halosB -> conv3(0).
            # conv1 pairs and conv3 pairs fill the PE between jp groups.
            def c1s(p):
                return lambda: (conv1_pair(p), swp(p))

            def c3(p):
                return lambda: conv3_pair(p)

            conv1_pair(0)
            swp(0)
            conv1_pair(1)
            swp(1)
            stage_q(0, cbs={0: c1s(6), 1: c1s(7), 2: c1s(2), 3: c1s(3)})
            halosA()
            stage_q(3, cbs={0: c1s(4), 1: c1s(5)})
            halosB()
            stage_q(1, cbs={1: c3(0), 3: c3(7)})
            stage_q(2, cbs={1: c3(1), 3: c3(2)})
            conv3_pair(3)
            conv3_pair(4, psCp, "ps2")
            conv3_pair(5)
            conv3_pair(6, psCp, "ps2")

    nc.compile()
    return nc


def _prep_weights(w_gk1, b_gk1, w_gk2, b_gk2, w_fuse, b_fuse):
    bf = ml_dtypes.bfloat16

    def conv_lhst(wc):
        l = np.empty((128, 9 * C), dtype=bf)
        for t in range(9):
            dy, dx = _tap_off(t)
            m = wc[:, :, dy, dx].T.astype(bf)  # [I, O] lhsT
            l[0:64, t * C:(t + 1) * C] = m
            l[64:128, t * C:(t + 1) * C] = m
        return l

    w1d = conv_lhst(np.asarray(w_gk1))
    w3d = conv_lhst(np.asarray(w_fuse))

    w2 = np.asarray(w_gk2).reshape(C * 9, C)
    w2d = np.empty((128, 9 * C), dtype=bf)
    for t in range(9):
        m = w2[t::9, :].T.astype(bf)
        w2d[0:64, t * C:(t + 1) * C] = m
        w2d[64:128, t * C:(t + 1) * C] = m

    b1 = np.asarray(b_gk1, np.float32)
    b3 = np.asarray(b_fuse, np.float32)
    b1d = np.concatenate([b1, b1]).reshape(128, 1)
    b3d = np.concatenate([b3, b3]).reshape(128, 1)
    b2 = np.asarray(b_gk2, np.float32).reshape(C, 9)
    b2d = np.concatenate([b2, b2], axis=0)
    wall = np.ascontiguousarray(np.concatenate([w1d, w2d, w3d], axis=1))
    ball = np.ascontiguousarray(
        np.concatenate([b1d, b2d, b3d], axis=1).astype(np.float32))
    return wall, ball


def _prep_pad(img):
    """[64,128,128] fp32 -> [128,66,130] bf16 padded, half-split."""
    bf = ml_dtypes.bfloat16
    b = np.asarray(img, np.float32).astype(bf)
    pad = np.zeros((128, PR, PW), dtype=bf)
    pad[0:64, 1:66, 1:129] = b[:, 0:65]       # h0: img rows 0..64
    pad[64:128, 0:65, 1:129] = b[:, 63:128]   # h1: img rows 63..127
    return pad


def prep_in_map(inputs, i):
    wall, ball = _prep_weights(
        inputs["w_gk1"], inputs["b_gk1"], inputs["w_gk2"],
        inputs["b_gk2"], inputs["w_fuse"], inputs["b_fuse"])
    x = np.asarray(inputs["x"], np.float32)
    y = np.asarray(inputs["y"], np.float32)
    return {"xpd": _prep_pad(x[i]), "ypd": _prep_pad(y[i]),
            "wall": wall, "ball": ball}


def post_out(out, inputs):
    return out


def kernel(x, y, w_gk1, b_gk1, w_gk2, b_gk2, w_fuse, b_fuse):
    from concourse.bass_utils import run_bass_kernel_spmd

    with _lock:
        if "nc" not in _cache:
            _cache["nc"] = build_nc()
    nc = _cache["nc"]

    wall, ball = _prep_weights(
        w_gk1, b_gk1, w_gk2, b_gk2, w_fuse, b_fuse)

    x = np.asarray(x, np.float32)
    y = np.asarray(y, np.float32)
    n = x.shape[0]
    assert n == 8, f"expected batch 8, got {n}"
    in_maps = []
    for i in range(n):
        in_maps.append({
            "xpd": _prep_pad(x[i]), "ypd": _prep_pad(y[i]),
            "wall": wall, "ball": ball,
        })
    res = run_bass_kernel_spmd(nc, in_maps, core_ids=list(range(n)))
    out = np.stack([res.results[i]["out"] for i in range(n)], axis=0)
    return post_out(out, {"x": x, "y": y})


# revision 26
# speedup vs baseline: 1.1357x; 1.1357x over previous
"""DepthDC3x3 dynamic depthwise conv — Trainium2 Bass kernel, 8 NeuronCores.

Per-sample pipeline (data-parallel over batch N=8 -> one sample per core):
  A  = conv3x3(y, w_gk1) + b_gk1                    # [64,128,128]
  K  = conv1x1(A, w_gk2) + b_gk2                    # [576,128,128] -> per-tap
  r  = sum_k K[:,k] * shift_k(x)                    # dynamic depthwise 3x3
  out= conv3x3(r, w_fuse) + b_fuse                  # [64,128,128]

Key design points (v7, "all-normal"):
  * Half-split layout: SBUF partitions = (channel 0..63) x (image half).
  * Inputs are pre-padded + pre-cast to bf16 on the HOST into the exact
    on-chip layout ([128 part, 66, 130]); chunked HWDGE loads (y on the
    sync ring, x on the scalar ring) feed compute directly.
  * conv1 window-pairs produce one normal + one halves-swapped ("inverted")
    PSUM tile -> a2 windows alternate normal/inverted. A cheap SBUF->SBUF
    swap DMA per pair builds a2s (the opposite orientation), so every
    window is available in BOTH orientations.
  * Stage C uses the normal-orientation source for psA quadrants and the
    inverted source for psB quadrants -> ALL conv2 outputs land normal.
    Single accumulation chain (9 muls + 8 adds) writes straight into the
    one padded accumulator rpn. No rpi, no xswp.
  * conv3 is a single 9-tap pass over rpn (half the v6 PE work), one ACT
    eviction, no DVE add.
  * bf16 matmuls, 2x2 tile_position quadrants -> 4x PE concurrency.
"""

import threading

import numpy as np
import ml_dtypes

import concourse.bacc as bacc
import concourse.mybir as mybir
from concourse.tile import TileContext

BF = mybir.dt.bfloat16
F32 = mybir.dt.float32

C = 64            # channels
HR = 64           # rows per half
W = 128           # image width
PW = W + 2        # padded width
PR = HR + 2       # padded rows per half
NWIN = 16         # windows per half (4 rows x 128 cols each)
WPX = 512
RW = 4            # rows per window
KS = 3

# row chunks of the padded-row index space (0..66)
CHUNKS = ((0, 18), (18, 34), (34, 50), (50, 66))


_lock = threading.Lock()
_cache = {}


def _tap_off(t):
    return divmod(t, KS)


def build_nc():
    nc = bacc.Bacc("TRN2", target_bir_lowering=False, debug=False)

    xp_d = nc.dram_tensor("xpd", [128, PR, PW], BF, kind="ExternalInput")
    yp_d = nc.dram_tensor("ypd", [128, PR, PW], BF, kind="ExternalInput")
    wall_d = nc.dram_tensor("wall", [128, 27 * C], BF, kind="ExternalInput")
    ball_d = nc.dram_tensor("ball", [128, 11], F32, kind="ExternalInput")
    out_d = nc.dram_tensor("out", [C, 2 * HR, W], F32, kind="ExternalOutput")

    with TileContext(nc) as tc:
        with (
            tc.tile_pool(name="big", bufs=1) as big,
            tc.tile_pool(name="wpool", bufs=1) as wpool,
            tc.tile_pool(name="kev", bufs=6) as kevp,
            tc.tile_pool(name="prod", bufs=6) as prodp,
            tc.tile_pool(name="accs", bufs=4) as accp,
            tc.tile_pool(name="stag", bufs=3) as stagp,
            tc.tile_pool(name="ps13", bufs=2, space="PSUM") as ps13p,
            tc.tile_pool(name="psC", bufs=2, space="PSUM") as psCp,
        ):
            ypad = big.tile([128, PR, PW], BF)
            xpad = big.tile([128, PR, PW], BF)
            a2 = big.tile([128, NWIN * WPX], BF)
            a2s = big.tile([128, NWIN * WPX], BF)
            rpn = big.tile([128, PR, PW], BF)

            wall = wpool.tile([128, 27 * C], BF)
            ball = wpool.tile([128, 11], F32)
            W2O, W3O = 9 * C, 18 * C

            def ldchunk(dst, src, k, q):
                r0, r1 = CHUNKS[k]
                q.dma_start(out=dst[:, r0:r1, :], in_=src[:, r0:r1, :])

            ident = mybir.ActivationFunctionType.Identity

            def conv9(src, woff, psLR, w0):
                """3x3 conv quadrants for adjacent window-pair (w0, w0+1).
                psLR [128,1024]: [:,0:512] win w0 normal; [:,512:1024] win
                w0+1 inverted."""
                r0, r1 = RW * w0, RW * (w0 + 1)
                for t in range(9):
                    dy, dx = _tap_off(t)
                    st, sp = (t == 0), (t == 8)
                    lhsL = wall[0:64, woff + t * C:woff + (t + 1) * C]
                    lhsH = wall[64:128, woff + t * C:woff + (t + 1) * C]
                    nc.tensor.matmul(psLR[0:64, 0:512], lhsL,
                                     src[0:64, r0 + dy:r0 + dy + 4, dx:dx + 128],
                                     start=st, stop=sp, tile_position=(0, 0),
                                     skip_group_check=True)
                    nc.tensor.matmul(psLR[64:128, 0:512], lhsH,
                                     src[64:128, r0 + dy:r0 + dy + 4, dx:dx + 128],
                                     start=st, stop=sp, tile_position=(64, 64),
                                     skip_group_check=True)
                    nc.tensor.matmul(psLR[64:128, 512:1024], lhsL,
                                     src[0:64, r1 + dy:r1 + dy + 4, dx:dx + 128],
                                     start=st, stop=sp, tile_position=(0, 64),
                                     skip_group_check=True)
                    nc.tensor.matmul(psLR[0:64, 512:1024], lhsH,
                                     src[64:128, r1 + dy:r1 + dy + 4, dx:dx + 128],
                                     start=st, stop=sp, tile_position=(64, 0),
                                     skip_group_check=True)

            def conv1_pair(p):
                w0 = 2 * p
                psLR = ps13p.tile([128, 2 * WPX], F32, tag="ps13",
                                  name=f"c1ps{p}")
                conv9(ypad, 0, psLR, w0)
                nc.scalar.activation(a2[:, w0 * WPX:(w0 + 2) * WPX],
                                     psLR[:, :], ident, bias=ball[:, 0:1])

            def swp(p):
                """halves-swapped copy of a2 windows (2p, 2p+1) -> a2s.
                gpsimd (SWDGE) ring: fires the moment the conv1 eviction
                lands, without queueing behind the y/x bulk loads."""
                sl = slice(2 * p * WPX, (2 * p + 2) * WPX)
                nc.sync.dma_start(out=a2s[0:64, sl], in_=a2[64:128, sl])
                nc.sync.dma_start(out=a2s[64:128, sl], in_=a2[0:64, sl])

            def x2win(xsrc, wA, dy, dx):
                """[128, 2, 4, 128] view covering same-parity windows wA,
                wA+2 at tap offset (dy, dx)."""
                r0 = RW * wA + dy
                return xsrc[:, r0:r0 + 12, dx:dx + 128].rearrange(
                    "p (b r) c -> p b r c", b=3)[:, 0::2]

            def stage_c(wA, cbs=None):
                """conv2 + dynamic multiply-sum for windows (wA, wA+2).
                All-normal: psA reads the normal-orientation source, psB the
                inverted one, so every tap's product is normal and a single
                chain accumulates into rpn. The last tap writes rpn in two
                per-window adds so dependent conv3 pairs unlock earlier.
                cbs = {jp: callback} sprinkles filler emission."""
                wB = wA + 2
                nsrc = a2 if wA % 2 == 0 else a2s
                isrc = a2s if wA % 2 == 0 else a2
                cnt = 0
                acc_t = None
                r0 = RW * wA

                for jp in range(5):
                    ta = 2 * jp
                    taps = (ta,) if ta == 8 else (ta, ta + 1)
                    psA = psCp.tile([128, 2 * WPX], F32, tag="ps2",
                                    name=f"psA{wA}_{jp}")
                    lhsA_l = wall[0:64, W2O + ta * C:W2O + (ta + 1) * C]
                    lhsA_h = wall[64:128, W2O + ta * C:W2O + (ta + 1) * C]
                    psB = None
                    if len(taps) == 2:
                        tb = ta + 1
                        psB = psCp.tile([128, 2 * WPX], F32, tag="ps2",
                                        name=f"psB{wA}_{jp}")
                        lhsB_l = wall[0:64, W2O + tb * C:W2O + (tb + 1) * C]
                        lhsB_h = wall[64:128, W2O + tb * C:W2O + (tb + 1) * C]
                    for wi, wv in ((0, wA), (1, wB)):
                        nwin = nsrc[:, wv * WPX:(wv + 1) * WPX]
                        sl = slice(wi * WPX, (wi + 1) * WPX)
                        nc.tensor.matmul(psA[0:64, sl], lhsA_l, nwin[0:64, :],
                                         tile_position=(0, 0),
                                         skip_group_check=True)
                        nc.tensor.matmul(psA[64:128, sl], lhsA_h,
                                         nwin[64:128, :],
                                         tile_position=(64, 64),
                                         skip_group_check=True)
                        if psB is not None:
                            iwin = isrc[:, wv * WPX:(wv + 1) * WPX]
                            nc.tensor.matmul(psB[64:128, sl], lhsB_l,
                                             iwin[0:64, :],
                                             tile_position=(0, 64),
                                             skip_group_check=True)
                            nc.tensor.matmul(psB[0:64, sl], lhsB_h,
                                             iwin[64:128, :],
                                             tile_position=(64, 0),
                                             skip_group_check=True)

                    for t in taps:
                        ps = psA if t == ta else psB
                        dy, dx = _tap_off(t)
                        xop = x2win(xpad, wA, dy, dx)
                        cnt += 1
                        first = (cnt == 1)
                        last = (cnt == 9)
                        if first:
                            acc_t = accp.tile([128, 2 * WPX], BF, tag="acc",
                                              name=f"ac{wA}")
                            tgt = acc_t
                        else:
                            tgt = prodp.tile([128, 2 * WPX], BF, tag="prod",
                                             name=f"pr{wA}_{t}")
                        tgt4 = tgt[:, :].rearrange("p (b r f) -> p b r f",
                                                   b=2, r=4)
                        kev = kevp.tile([128, 2 * WPX], BF, tag="kev",
                                        name=f"kv{wA}_{t}")
                        nc.scalar.activation(kev[:, :], ps[:, :], ident,
                                             bias=ball[:, 1 + t:2 + t])
                        nc.vector.tensor_mul(
                            out=tgt4,
                            in0=kev[:, :].rearrange(
                                "p (b r f) -> p b r f", b=2, r=4),
                            in1=xop)
                        if not first:
                            if last:
                                for wi, wv in ((0, wA), (1, wB)):
                                    rw = RW * wv
                                    sl = slice(wi * WPX, (wi + 1) * WPX)
                                    nc.vector.tensor_add(
                                        out=rpn[:, rw + 1:rw + 5, 1:129],
                                        in0=acc_t[:, sl].rearrange(
                                            "p (r f) -> p r f", r=4),
                                        in1=tgt[:, sl].rearrange(
                                            "p (r f) -> p r f", r=4))
                            else:
                                nc.vector.tensor_add(out=acc_t[:, :],
                                                     in0=acc_t[:, :],
                                                     in1=tgt[:, :])
                    if cbs and jp in cbs:
                        cbs[jp]()

            def halosA():
                # available after stage_c(0): rpn h0 bottom halo row.
                nc.sync.dma_start(out=rpn[0:64, 65:66, :],
                                  in_=rpn[64:128, 1:2, :])

            def halosB():
                # available after stage_c(13): rpn h1 top halo row
                nc.sync.dma_start(out=rpn[64:128, 0:1, :],
                                  in_=rpn[0:64, 64:65, :])

            # conv3: single 9-tap pass over rpn for windows (2p, 2p+1),
            # one ACT eviction, 4 stores.
            c3ps = {}

            def c3n(p, pool=ps13p, tg="ps13"):
                psN = pool.tile([128, 2 * WPX], F32, tag=tg, name=f"c3n{p}")
                conv9(rpn, W3O, psN, 2 * p)
                c3ps[p] = psN

            def c3fin(p, dve_evict=False):
                psN = c3ps.pop(p)
                w0 = 2 * p
                st_t = stagp.tile([128, 2 * WPX], F32, tag="stag",
                                  name=f"st{p}")
                if dve_evict:
                    # tail pairs: DVE tensor_scalar (keeps the b3 bias)
                    # while ACT drains the last stage_c kev evictions.
                    nc.vector.tensor_scalar_add(out=st_t[:, :],
                                                in0=psN[:, :],
                                                scalar1=ball[:, 10:11])
                else:
                    nc.scalar.activation(st_t[:, :], psN[:, :], ident,
                                         bias=ball[:, 10:11])
                ra, rb = RW * w0, RW * (w0 + 1)
                # last pair: split stores across both rings to halve the
                # final drain (scalar engine is idle by then)
                q2 = nc.scalar if p == 7 else nc.sync
                nc.sync.dma_start(out=out_d[:, ra:ra + 4, :],
                                  in_=st_t[0:64, 0:512])
                q2.dma_start(out=out_d[:, HR + ra:HR + ra + 4, :],
                             in_=st_t[64:128, 0:512])
                nc.sync.dma_start(out=out_d[:, HR + rb:HR + rb + 4, :],
                                  in_=st_t[0:64, 512:1024])
                q2.dma_start(out=out_d[:, rb:rb + 4, :],
                             in_=st_t[64:128, 512:1024])

            def conv3_pair(p, pool=ps13p, tg="ps13", dve_evict=False):
                c3n(p, pool, tg)
                c3fin(p, dve_evict)

            # ---- emission schedule ----
            # edge memsets FIRST: emitting them before the warm-up matmuls
            # avoids a false flat-range dependency on the warm-up's rpn
            # reads that would delay them by several us.
            nc.vector.memset(rpn[0:64, 0:1, :], 0.0)
            nc.vector.memset(rpn[64:128, 65:66, :], 0.0)
            nc.vector.memset(rpn[:, :, 0:1], 0.0)
            nc.vector.memset(rpn[:, :, 129:130], 0.0)

            # PE HAM warm-up: dummy matmuls spread across all 4 quadrants
            # (4-way concurrent) flip the clock gate to 8/8 during the load
            # window; results are discarded. Source rows are zeroed first so
            # the reads are initialized (keeps CoreSim usable).
            nc.vector.memset(rpn[:, 20:26, :], 0.0)
            nc.vector.memset(rpn[:, 30:36, :], 0.0)
            psW = psCp.tile([128, 2 * WPX], F32, tag="ps2", name="warm")
            wq = (
                (psW[0:64, 0:512], rpn[0:64, 20:21, 1:65],
                 rpn[0:64, 22:26, 1:129], (0, 0)),
                (psW[64:128, 0:512], rpn[64:128, 20:21, 1:65],
                 rpn[64:128, 22:26, 1:129], (64, 64)),
                (psW[64:128, 512:1024], rpn[0:64, 30:31, 1:65],
                 rpn[0:64, 32:36, 1:129], (0, 64)),
                (psW[0:64, 512:1024], rpn[64:128, 30:31, 1:65],
                 rpn[64:128, 32:36, 1:129], (64, 0)),
            )
            for i in range(32):
                o, l, r, tp = wq[i % 4]
                nc.tensor.matmul(o, l, r, tile_position=tp,
                                 skip_group_check=True)

            # weights on the scalar ring: they load in parallel with y0a
            # instead of serializing between the y chunks on the sync ring
            nc.scalar.dma_start(out=wall[:], in_=wall_d[:])
            nc.scalar.dma_start(out=ball[:], in_=ball_d[:])
            nc.sync.dma_start(out=ypad[:, 0:10, :], in_=yp_d[:, 0:10, :])
            nc.sync.dma_start(out=ypad[:, 10:18, :], in_=yp_d[:, 10:18, :])
            # ACT spline-table prewarm: a tiny Identity right after ball
            # lands absorbs the one-time ~2.7us ACT_TABLE_LOAD during the
            # load window instead of on the first real eviction.
            prew = stagp.tile([128, 2 * WPX], F32, tag="stag", name="prew")
            nc.scalar.activation(prew[:, 0:1], ball[:, 0:1], ident)

            # conv1 consumes pairs 0,1,6,7 first (y chunks 2,3 before 1);
            # x chunks follow the stage_c order 0,13,1,12,...
            for k in (2, 3, 1):
                ldchunk(ypad, yp_d, k, nc.sync)
            for k in (0, 3, 1, 2):
                ldchunk(xpad, xp_d, k, nc.scalar)

            # conv1 pairs 6,7 run early so stage_c(13) — which gates
            # halosB -> conv3(0) — can run 2nd with ample swap slack;
            # conv3 pairs then spread mid-phase, leaving a 3-pair tail.
            conv1_pair(0)
            swp(0)
            conv1_pair(1)
            swp(1)
            conv1_pair(6)
            swp(6)
            conv1_pair(7)
            swp(7)
            stage_c(0)
            halosA()
            conv1_pair(2)
            swp(2)
            stage_c(13)
            halosB()
            conv1_pair(3)
            swp(3)
            stage_c(1)
            conv1_pair(4)
            swp(4)
            stage_c(12)
            conv1_pair(5)
            swp(5)
            stage_c(4)
            conv3_pair(0)
            stage_c(5)
            conv3_pair(1)
            conv3_pair(7)
            stage_c(8)
            conv3_pair(2)
            conv3_pair(3)
            stage_c(9)
            conv3_pair(4, dve_evict=True)
            conv3_pair(5, psCp, "ps2", dve_evict=True)
            conv3_pair(6, dve_evict=True)

    nc.compile()
    return nc


def _prep_weights(w_gk1, b_gk1, w_gk2, b_gk2, w_fuse, b_fuse):
    bf = ml_dtypes.bfloat16

    def conv_lhst(wc):
        l = np.empty((128, 9 * C), dtype=bf)
        for t in range(9):
            dy, dx = _tap_off(t)
            m = wc[:, :, dy, dx].T.astype(bf)  # [I, O] lhsT
            l[0:64, t * C:(t + 1) * C] = m
            l[64:128, t * C:(t + 1) * C] = m
        return l

    w1d = conv_lhst(np.asarray(w_gk1))
    w3d = conv_lhst(np.asarray(w_fuse))

    w2 = np.asarray(w_gk2).reshape(C * 9, C)
    w2d = np.empty((128, 9 * C), dtype=bf)
    for t in range(9):
        m = w2[t::9, :].T.astype(bf)
        w2d[0:64, t * C:(t + 1) * C] = m
        w2d[64:128, t * C:(t + 1) * C] = m

    b1 = np.asarray(b_gk1, np.float32)
    b3 = np.asarray(b_fuse, np.float32)
    b1d = np.concatenate([b1, b1]).reshape(128, 1)
    b3d = np.concatenate([b3, b3]).reshape(128, 1)
    b2 = np.asarray(b_gk2, np.float32).reshape(C, 9)
    b2d = np.concatenate([b2, b2], axis=0)
    wall = np.ascontiguousarray(np.concatenate([w1d, w2d, w3d], axis=1))
    ball = np.ascontiguousarray(
        np.concatenate([b1d, b2d, b3d], axis=1).astype(np.float32))
    return wall, ball


def _prep_pad(img):
    """[64,128,128] fp32 -> [128,66,130] bf16 padded, half-split."""
    bf = ml_dtypes.bfloat16
    b = np.asarray(img, np.float32).astype(bf)
    pad = np.zeros((128, PR, PW), dtype=bf)
    pad[0:64, 1:66, 1:129] = b[:, 0:65]       # h0: img rows 0..64
    pad[64:128, 0:65, 1:129] = b[:, 63:128]   # h1: img rows 63..127
    return pad


def prep_in_map(inputs, i):
    wall, ball = _prep_weights(
        inputs["w_gk1"], inputs["b_gk1"], inputs["w_gk2"],
        inputs["b_gk2"], inputs["w_fuse"], inputs["b_fuse"])
    x = np.asarray(inputs["x"], np.float32)
    y = np.asarray(inputs["y"], np.float32)
    return {"xpd": _prep_pad(x[i]), "ypd": _prep_pad(y[i]),
            "wall": wall, "ball": ball}


def post_out(out, inputs):
    return out


def kernel(x, y, w_gk1, b_gk1, w_gk2, b_gk2, w_fuse, b_fuse):
    from concourse.bass_utils import run_bass_kernel_spmd

    with _lock:
        if "nc" not in _cache:
            _cache["nc"] = build_nc()
    nc = _cache["nc"]

    wall, ball = _prep_weights(
        w_gk1, b_gk1, w_gk2, b_gk2, w_fuse, b_fuse)

    x = np.asarray(x, np.float32)
    y = np.asarray(y, np.float32)
    n = x.shape[0]
    assert n == 8, f"expected batch 8, got {n}"
    in_maps = []
    for i in range(n):
        in_maps.append({
            "xpd": _prep_pad(x[i]), "ypd": _prep_pad(y[i]),
            "wall": wall, "ball": ball,
        })
    res = run_bass_kernel_spmd(nc, in_maps, core_ids=list(range(n)))
    out = np.stack([res.results[i]["out"] for i in range(n)], axis=0)
    return post_out(out, {"x": x, "y": y})
